# revision 13
# baseline (speedup 1.0000x reference)
"""TT-adapter linear kernel for TRN2, data-parallel over batch on 8 NeuronCores.

Math: out = x @ W.T + b + ALPHA * TT(x), where TT is a tensor-train
factorized linear map (6 small cores).  TT is linear in x, so the module
collapses to a single matmul with a merged weight:

    T  = TT-matrix reconstruction (1024x1024, ~17 MFLOP, folded on host)
    Wc = W + ALPHA * T
    out = x @ Wc.T + b

The 34 GFLOP batched matmul runs on device in bf16 (f32 PSUM accumulation),
one batch element per NeuronCore, no collectives.  Raw bacc (manual
semaphores), measured ~75-78 us on silicon vs a ~55 us pure-TensorE
roofline.

Host layouts (per core, P=128 partitions, contraction dim on partitions):
    xt  bf16 [8, 128, 2048]  xt[d, p, s]   = x[b, s, 128*d + p]
    wt  bf16 [8, 128, 1024]  wt[d, p, o]   = Wc[o, 128*d + p]
    bi  f32  [128, 8]        bi[p, oo]     = b[128*oo + p]
    out f32  [8, 128, 2048]  out[oo, p, s] = result[b, s, 128*oo + p]

Schedule per core (group g = (o, sc), o = g//4, sc = g%4, bank/slot = g%8):
  SP:  (w_d, x_d) DMAs interleaved + bias, then output DMAs g=0..29 gated
       on evictions, final wait for all out-DMA completions.
  PE:  20 short HAM-warm-up matmuls in the preamble/input-latency window, then
       phase 1 = groups 0..7 d-outer staircase (matmuls start as (w_d, x_d)
       arrive), phase 2 = groups 8..31 sequential d-inner, gated on bank
       eviction.  Per-d input semaphores (HWDGE completions are unordered).
  ACT: 32 evictions (PSUM -> SBUF + per-partition bias add) + the last two
       output DMAs shipped directly (skips the SP semaphore hop on the tail).
"""

import numpy as np
import ml_dtypes
from contextlib import ExitStack

import concourse.bass as bass  # noqa: F401
import concourse.mybir as mybir
from concourse import bacc
from concourse.bass_utils import run_bass_kernel_spmd

ALPHA = 16.0
B, S, D = 8, 2048, 1024
P = 128
DO = D // P
OO = D // P
SCH = 512
NS = S // SCH
NG = OO * NS        # 32 groups
NBANK = 8
NSLOT = 8

_NC = None


def _build_nc():
    nc = bacc.Bacc("TRN2", target_bir_lowering=False, debug=False)
    xt = nc.declare_dram_parameter("xt", [DO, P, S], mybir.dt.bfloat16, isOutput=False)
    wt = nc.declare_dram_parameter("wt", [DO, P, D], mybir.dt.bfloat16, isOutput=False)
    bi = nc.declare_dram_parameter("bi", [P, OO], mybir.dt.float32, isOutput=False)
    out = nc.declare_dram_parameter("out", [OO, P, S], mybir.dt.float32, isOutput=True)

    with ExitStack() as ctx:
        block = ctx.enter_context(nc.Block())
        # HWDGE completions on one queue are NOT ordered across DMAs, so a
        # single cumulative input semaphore is racy — use one sem per d-tile
        # (w_d + x_d -> 32) plus one for the bias.
        s_wx = [ctx.enter_context(nc.semaphore(f"s_wx{d}")) for d in range(DO)]
        s_bias = ctx.enter_context(nc.semaphore("s_bias"))
        s_mm = ctx.enter_context(nc.semaphore("s_mm"))
        s_ev = ctx.enter_context(nc.semaphore("s_ev"))
        # per-staging-slot out-DMA completion sems (same ordering concern)
        s_slot = [ctx.enter_context(nc.semaphore(f"s_slot{k}")) for k in range(NSLOT)]
        bias_sb = ctx.enter_context(nc.sbuf_tensor("bias_sb", [P, OO], mybir.dt.float32))
        w_sb = ctx.enter_context(nc.sbuf_tensor("w_sb", [P, DO, D], mybir.dt.bfloat16))
        x_sb = ctx.enter_context(nc.sbuf_tensor("x_sb", [P, DO, S], mybir.dt.bfloat16))
        ot_sb = ctx.enter_context(nc.sbuf_tensor("ot_sb", [P, NSLOT, SCH], mybir.dt.float32))
        ps = [ctx.enter_context(nc.psum_tensor(f"ps{b}", [P, SCH], mybir.dt.float32))
              for b in range(NBANK)]

        @block.sync
        def _(sync: bass.BassEngine):
            for d in range(DO):
                sync.dma_start(out=w_sb[:, d, :], in_=wt[d]).then_inc(s_wx[d], 16)
                sync.dma_start(out=x_sb[:, d, :], in_=xt[d]).then_inc(s_wx[d], 16)
            # bias is only needed by the first eviction (~25us in)
            sync.dma_start(out=bias_sb[:, :], in_=bi[:, :]).then_inc(s_bias, 16)
            for g in range(NG - 2):
                o, sc = g // NS, g % NS
                sync.wait_ge(s_ev, g + 1)
                sync.dma_start(
                    out=out[o, :, sc * SCH:(sc + 1) * SCH],
                    in_=ot_sb[:, g % NSLOT, :],
                ).then_inc(s_slot[g % NSLOT], 16)
            for k in range(NSLOT):
                sync.wait_ge(s_slot[k], 16 * (NG // NSLOT))

        @block.tensor
        def _(tensor: bass.BassEngine):
            # HAM warm-up: dummy matmuls on whatever is in SBUF during the
            # otherwise-idle preamble/input-latency window, so the PE clock
            # gate is at 8/8 when real matmuls start.  Results land in bank 0
            # and are discarded (group 0 re-starts it with start=True).
            for _ in range(20):
                tensor.matmul(
                    ps[0][:, 0:256],
                    w_sb[:, 0, 0:P],
                    x_sb[:, 0, 0:256],
                    start=True,
                    stop=True,
                )
            # phase 1: groups 0..7 on banks 0..7, d-outer staircase
            for i, d in enumerate(range(DO)):
                tensor.wait_ge(s_wx[d], 32)
                for g in range(NBANK):
                    o, sc = g // NS, g % NS
                    mmi = tensor.matmul(
                        ps[g][:, :],
                        w_sb[:, d, o * P:(o + 1) * P],
                        x_sb[:, d, sc * SCH:(sc + 1) * SCH],
                        start=(i == 0),
                        stop=(i == DO - 1),
                    )
                    if i == DO - 1:
                        mmi.then_inc(s_mm, 1)
            # phase 2: groups 8..31 sequential, d-inner
            for g in range(NBANK, NG):
                o, sc = g // NS, g % NS
                tensor.wait_ge(s_ev, g - NBANK + 1)
                for d in range(DO):
                    mmi = tensor.matmul(
                        ps[g % NBANK][:, :],
                        w_sb[:, d, o * P:(o + 1) * P],
                        x_sb[:, d, sc * SCH:(sc + 1) * SCH],
                        start=(d == 0),
                        stop=(d == DO - 1),
                    )
                    if d == DO - 1:
                        mmi.then_inc(s_mm, 1)

        @block.scalar
        def _(scalar: bass.BassEngine):
            scalar.wait_ge(s_bias, 16)
            for g in range(NG):
                o, sc = g // NS, g % NS
                scalar.wait_ge(s_mm, g + 1)
                if g >= NSLOT:
                    scalar.wait_ge(s_slot[g % NSLOT], 16 * (g // NSLOT))
                scalar.add(
                    ot_sb[:, g % NSLOT, :], ps[g % NBANK][:, :], bias_sb[:, o:o + 1]
                ).then_inc(s_ev, 1)
                if g >= NG - 2:
                    # last outputs: ACT (also HWDGE) ships them directly,
                    # skipping the SP semaphore hop on the critical tail
                    scalar.dma_start(
                        out=out[o, :, sc * SCH:(sc + 1) * SCH],
                        in_=ot_sb[:, g % NSLOT, :],
                    ).then_inc(s_slot[g % NSLOT], 16)

    nc.compile()
    return nc


def _get_nc():
    global _NC
    if _NC is None:
        _NC = _build_nc()
    return _NC


def _merged_weight_T(W, b, core0, core1, core2, core3, core4, core5):
    f8 = np.float64
    A = core0[0].astype(f8)
    Bm = np.einsum('ap,pbq->abq', A, core1.astype(f8))
    C = np.einsum('abq,qcr->abcr', Bm, core2.astype(f8))
    Phi = C.transpose(2, 1, 0, 3).reshape(D, 8)
    Dn = np.einsum('paq,qbr->pabr', core3.astype(f8), core4.astype(f8))
    E = np.einsum('pabq,qc->pabc', Dn, core5[:, :, 0].astype(f8))
    Psi = E.reshape(8, D)
    WcT = W.T.astype(f8) + ALPHA * (Phi @ Psi)
    return WcT.astype(np.float32)


def _prep_in_maps(x, W, b, core0, core1, core2, core3, core4, core5):
    WcT = _merged_weight_T(W, b, core0, core1, core2, core3, core4, core5)
    wt = WcT.reshape(DO, P, D).astype(ml_dtypes.bfloat16)
    bi = np.ascontiguousarray(b.reshape(OO, P).T).astype(np.float32)
    in_maps = []
    for bb in range(B):
        xt = x[bb].T.reshape(DO, P, S).astype(ml_dtypes.bfloat16)
        in_maps.append({"xt": xt, "wt": wt, "bi": bi})
    return in_maps


def _gather(results):
    outs = []
    for bb in range(B):
        o = np.asarray(results[bb]["out"])
        outs.append(o.transpose(2, 0, 1).reshape(S, D))
    return np.ascontiguousarray(np.stack(outs)).astype(np.float32)


def run(inputs, **spmd_kwargs):
    inputs = {k: np.asarray(v) for k, v in inputs.items()}
    in_maps = _prep_in_maps(**inputs)
    nc = _get_nc()
    res = run_bass_kernel_spmd(nc, in_maps, core_ids=list(range(B)), **spmd_kwargs)
    return _gather(res.results), res


def kernel(x, W, b, core0, core1, core2, core3, core4, core5):
    out, _ = run(dict(x=x, W=W, b=b, core0=core0, core1=core1, core2=core2,
                      core3=core3, core4=core4, core5=core5))
    return out
